# revision 1
# baseline (speedup 1.0000x reference)
"""Trainium2 Bass kernel for LoRA self-attention — v3: paired-scores lanes.

Like kernel2 (overlap-restructured, see its docstring) plus:

* Attention runs in "blocks" of 1-2 lanes; a lane = (head, 512-wide q-stripe).
  A pair block puts head A (qkT partitions 0:64) in lane 0 and head B
  (partitions 64:128) in lane 1: their scores matmuls target disjoint PE
  row-groups, so the hardware runs them concurrently (tile_position derives
  from base_partition) — scores cost drops ~2x for paired heads.
* One exp ACTIVATE covers the whole block ([128, lanes*512]), so the ACT
  instruction count (the exp overhead) stays the same as v2.
* h4 (no pair partner) runs as q 0:1024 with two q-stripe lanes, then two
  single-lane 512 blocks ordered so the final output-projection quanta for
  the last three q-stripes become in-schedule fillers instead of pure tail.
* PSUM: scores 2x[128,1024] + ctx 2x[128,512] + proj 2x[128,512] = 8 banks.
"""

import sys

if "/opt/trn_rl_repo" not in sys.path:
    sys.path.insert(0, "/opt/trn_rl_repo")

from contextlib import ExitStack

import ml_dtypes
import numpy as np

import concourse.bass as bass
import concourse.tile as tile
from concourse import bacc, mybir
from concourse.bass_utils import run_bass_kernel_spmd

BF16 = mybir.dt.bfloat16
F32 = mybir.dt.float32
NPBF16 = ml_dtypes.bfloat16

D = 64
H_LOC = 5
N_CORES = 8

Q_LOC = {0: (0, 0), 1: (0, 64), 2: (2, 0), 3: (2, 64), 4: (4, 0)}
K_LOC = {0: (1, 0), 1: (1, 64), 2: (3, 0), 3: (3, 64), 4: (5, 0)}


def build_program(S=2048, C=1280, repeat=1):
    assert S % 1024 == 0 and C % 128 == 0
    CK = C // 128
    SM = S // 128
    SK = S // 128
    NS4 = S // 512

    nc = bacc.Bacc("TRN2", target_bir_lowering=False, debug=False)

    xT_d = nc.dram_tensor("xT", [C, S], BF16, kind="ExternalInput").ap()
    wqk_d = nc.dram_tensor("wqk", [C, 640], BF16, kind="ExternalInput").ap()
    wvT_d = nc.dram_tensor("wvT", [C, H_LOC * D], BF16, kind="ExternalInput").ap()
    woT_d = nc.dram_tensor("woT", [384, C], BF16, kind="ExternalInput").ap()
    out_d = nc.dram_tensor("out_part", [S, C], F32, kind="ExternalOutput").ap()

    EXP = mybir.ActivationFunctionType.Exp
    MULT = mybir.AluOpType.mult

    with tile.TileContext(nc) as tc, ExitStack() as ctx:
        persist = ctx.enter_context(tc.tile_pool(name="persist", bufs=1))
        psp = ctx.enter_context(tc.tile_pool(name="ps", bufs=2, space="PSUM"))
        ptp = ctx.enter_context(tc.tile_pool(name="ptp", bufs=4))
        unp = ctx.enter_context(tc.tile_pool(name="unp", bufs=4))
        smallp = ctx.enter_context(tc.tile_pool(name="small", bufs=4))
        outp = ctx.enter_context(tc.tile_pool(name="osb", bufs=2))
        dramp = ctx.enter_context(tc.tile_pool(name="scratch", bufs=2, space="DRAM"))

        xT_sb = persist.tile([128, CK, S], BF16, tag="xT")
        wqk_sb = persist.tile([128, CK, 640], BF16, tag="wqk")
        wvT_sb = persist.tile([128, CK, H_LOC * D], BF16, tag="wvT")
        woT_sb = persist.tile([128, 3, C], BF16, tag="woT")
        qkT_sb = persist.tile([128, 6, S], BF16, tag="qkT")
        v_sb = persist.tile([128, SM, H_LOC, D + 1], BF16, tag="vsb")
        ctxT_sb = persist.tile([128, 3, S], BF16, tag="ctxT")

        def emit_body(rep):
            for s4 in range(NS4):
                nc.sync.dma_start(
                    xT_sb[:, :, s4 * 512 : (s4 + 1) * 512],
                    xT_d[:, s4 * 512 : (s4 + 1) * 512].rearrange(
                        "(o p) n -> p o n", p=128
                    ),
                )
            nc.gpsimd.dma_start(wqk_sb[:], wqk_d.rearrange("(o p) n -> p o n", p=128))
            nc.gpsimd.dma_start(wvT_sb[:], wvT_d.rearrange("(o p) n -> p o n", p=128))
            nc.gpsimd.dma_start(woT_sb[:], woT_d.rearrange("(o p) n -> p o n", p=128))

            nc.vector.memset(v_sb[:, :, :, D : D + 1], 1.0)
            if rep == 0:
                nc.vector.memset(ctxT_sb[64:128, 2, :], 0.0)

            # ---- PE work quanta -----------------------------------------
            def a1_quantum(f, s4):
                ps = psp.tile([128, 512], F32, tag="proj", name=f"a1ps_{f}_{s4}")
                for c in range(CK):
                    nc.tensor.matmul(
                        ps[:],
                        lhsT=wqk_sb[:, c, f * 128 : (f + 1) * 128],
                        rhs=xT_sb[:, c, s4 * 512 : (s4 + 1) * 512],
                        start=(c == 0),
                        stop=(c == CK - 1),
                    )
                nc.vector.tensor_copy(
                    out=qkT_sb[:, f, s4 * 512 : (s4 + 1) * 512], in_=ps[:]
                )
                if f == 4:
                    # realign k4 to partitions 0:64 of chunk 5 via DRAM
                    # bounce on the SP ring, which carries only small
                    # transfers (big output stores ride the gpsimd ring)
                    kscr = dramp.tile([64, 512], BF16, tag="kscr", name=f"kscr_{s4}")
                    nc.sync.dma_start(
                        kscr[:], qkT_sb[64:128, 4, s4 * 512 : (s4 + 1) * 512]
                    )
                    nc.sync.dma_start(
                        qkT_sb[0:64, 5, s4 * 512 : (s4 + 1) * 512], kscr[:]
                    )

            def a2_quantum(m):
                ps = psp.tile([128, 512], F32, tag="proj", name=f"a2ps_{m}")
                for c in range(CK):
                    nc.tensor.matmul(
                        ps[:, 0 : H_LOC * D],
                        lhsT=xT_sb[:, c, m * 128 : (m + 1) * 128],
                        rhs=wvT_sb[:, c, :],
                        start=(c == 0),
                        stop=(c == CK - 1),
                    )
                nc.vector.tensor_copy(
                    out=v_sb[:, m, :, 0:D],
                    in_=ps[:, 0 : H_LOC * D].rearrange("p (h d) -> p h d", h=H_LOC),
                )

            def op_quantum(m):
                out_sb = outp.tile([128, C], F32, tag="osb", name=f"osb_{m}")
                for col0 in range(0, C, 512):
                    w = min(512, C - col0)
                    ps = psp.tile([128, 512], F32, tag="proj", name=f"ops_{m}_{col0}")
                    for j in range(3):
                        nc.tensor.matmul(
                            ps[:, 0:w],
                            lhsT=ctxT_sb[:, j, m * 128 : (m + 1) * 128],
                            rhs=woT_sb[:, j, col0 : col0 + w],
                            start=(j == 0),
                            stop=(j == 2),
                        )
                    nc.vector.tensor_copy(
                        out=out_sb[:, col0 : col0 + w], in_=ps[:, 0:w]
                    )
                # output stores ride the gpsimd ring (idle after weight loads)
                nc.gpsimd.dma_start(out_d[m * 128 : (m + 1) * 128, :], out_sb[:])

            # ---- attention block: 1-2 lanes, 16 sk slots ----------------
            def attention_block(lanes, fillers, bname):
                """lanes: list of (head, q0) with q0 a 512-aligned offset."""
                W = 512 * len(lanes)
                ctxs = [
                    psp.tile(
                        [128, 512], F32, tag="ctx", name=f"ctx_{bname}_{li}"
                    )
                    for li in range(len(lanes))
                ]
                nf = len(fillers)
                fi = 0
                for sk in range(SK):
                    sc = psp.tile([128, 1024], F32, tag="sc", name=f"sc_{bname}_{sk}")
                    for li, (h, q0) in enumerate(lanes):
                        qc, qo = Q_LOC[h]
                        kc, ko = K_LOC[h]
                        nc.tensor.matmul(
                            sc[:, li * 512 : (li + 1) * 512],
                            lhsT=qkT_sb[ko : ko + D, kc, sk * 128 : (sk + 1) * 128],
                            rhs=qkT_sb[qo : qo + D, qc, q0 : q0 + 512],
                            start=True,
                            stop=True,
                        )
                    pt = ptp.tile([128, 1024], BF16, tag="pt", name=f"pt_{bname}_{sk}")
                    nc.scalar.activation(pt[:, 0:W], sc[:, 0:W], EXP)
                    fto = (sk + 1) * nf // SK
                    while fi < fto:
                        fillers[fi]()
                        fi += 1
                    for li, (h, q0) in enumerate(lanes):
                        nc.tensor.matmul(
                            ctxs[li][0 : D + 1, :],
                            lhsT=v_sb[:, sk, h, :],
                            rhs=pt[:, li * 512 : (li + 1) * 512],
                            start=(sk == 0),
                            stop=(sk == SK - 1),
                        )
                for li, (h, q0) in enumerate(lanes):
                    un = unp.tile([128, 512], F32, tag="un", name=f"un_{bname}_{li}")
                    nc.vector.tensor_copy(
                        out=un[0 : D + 1, :], in_=ctxs[li][0 : D + 1, :]
                    )
                    rec = smallp.tile([1, 512], F32, tag="rec", name=f"rec_{bname}_{li}")
                    nc.vector.reciprocal(rec[:], un[D : D + 1, :])
                    # normalize bounce on the DVE ring: it gates ctxT (and so
                    # the output-projection fillers) and must not serialize
                    # behind output stores or input loads on other rings
                    scr = dramp.tile([1, 512], F32, tag="scr", name=f"scr_{bname}_{li}")
                    nc.sync.dma_start(scr[:], rec[:])
                    bc = smallp.tile([64, 512], F32, tag="bc", name=f"bc_{bname}_{li}")
                    nc.sync.dma_start(bc[:], scr[:].to_broadcast((64, 512)))
                    jc, po = h // 2, (h % 2) * 64
                    nc.vector.tensor_tensor(
                        out=ctxT_sb[po : po + D, jc, q0 : q0 + 512],
                        in0=un[0:D, :],
                        in1=bc[:],
                        op=MULT,
                    )

            # ---- schedule ----------------------------------------------
            a1_quantum(0, 0)
            a1_quantum(1, 0)
            a2_quantum(0)
            a2_quantum(1)
            a2_quantum(2)
            a2_quantum(3)

            A1 = lambda f, s: (lambda: a1_quantum(f, s))
            A2 = lambda m: (lambda: a2_quantum(m))
            OP = lambda m: (lambda: op_quantum(m))

            blocks = [
                ([(0, 0), (1, 0)],
                 [A1(1, 1), A1(1, 2), A1(1, 3)]
                 + [A2(m) for m in range(4, 16)] + [A1(0, 1)], "p0s0"),
                ([(0, 512), (1, 512)],
                 [A1(0, 2), A1(3, 0), A1(3, 1), A1(3, 2)], "p0s1"),
                ([(0, 1024), (1, 1024)],
                 [A1(0, 3), A1(3, 3), A1(2, 0), A1(2, 1)], "p0s2"),
                ([(2, 0), (3, 0)], [A1(2, 2), A1(2, 3), A1(4, 0)], "p1s0"),
                ([(2, 512), (3, 512)], [A1(4, 1), A1(4, 2), A1(4, 3)], "p1s1"),
                ([(4, 0), (4, 512)], [], "h4a"),
                # q 0:1024 done for all heads -> op m0..7 ready
                ([(2, 1024), (3, 1024)], [OP(0), OP(1), OP(2)], "p1s2"),
                ([(4, 1024)], [OP(3), OP(4)], "h4b"),
                # q 1024:1536 done -> op m8..11 ready after this point
                ([(0, 1536), (1, 1536)], [OP(5), OP(6), OP(7)], "p0s3"),
                ([(4, 1536)], [OP(8), OP(9)], "h4c"),
                ([(2, 1536), (3, 1536)], [OP(10), OP(11)], "p1s3"),
            ]
            for lanes, fillers, bname in blocks:
                attention_block(lanes, fillers, bname)
            for m in range(12, 16):
                op_quantum(m)

        for rep in range(repeat):
            emit_body(rep)

    nc.compile()
    return nc


def make_core_inputs(x, Wq_eff, Wk_eff, Wv_eff, Wo_eff):
    B, S, C = x.shape
    in_maps = []
    xT16 = [np.ascontiguousarray(x[b].T).astype(NPBF16) for b in range(B)]
    for core in range(N_CORES):
        b, g = core // 4, core % 4
        r0 = g * H_LOC * D
        qf = Wq_eff[r0 : r0 + H_LOC * D]
        kf = Wk_eff[r0 : r0 + H_LOC * D]
        vf = Wv_eff[r0 : r0 + H_LOC * D]
        wqk = np.concatenate(
            [
                qf[0:128], kf[0:128],
                qf[128:256], kf[128:256],
                qf[256:320], kf[256:320],
            ],
            axis=0,
        ).T
        wvT = vf.T
        woT = np.concatenate(
            [Wo_eff[:, r0 : r0 + H_LOC * D].T, np.zeros((D, C), np.float32)], axis=0
        )
        in_maps.append(
            {
                "xT": xT16[b],
                "wqk": np.ascontiguousarray(wqk).astype(NPBF16),
                "wvT": np.ascontiguousarray(wvT).astype(NPBF16),
                "woT": np.ascontiguousarray(woT).astype(NPBF16),
            }
        )
    return in_maps


def fold_weights(Wq, Wk, Wv, Wo, Aq, Bq, Ak, Bk, Av, Bv, Ao, Bo):
    scale = 1.0 / np.sqrt(np.float32(D))
    Wq_eff = (Wq + Bq @ Aq) * scale
    Wk_eff = Wk + Bk @ Ak
    Wv_eff = Wv + Bv @ Av
    Wo_eff = Wo + Bo @ Ao
    return Wq_eff, Wk_eff, Wv_eff, Wo_eff


_NC_CACHE = {}


def _get_program(S, C):
    key = (S, C)
    if key not in _NC_CACHE:
        _NC_CACHE[key] = build_program(S, C)
    return _NC_CACHE[key]


def kernel(**inputs):
    inputs = {k: np.asarray(v, np.float32) for k, v in inputs.items()}
    x = inputs["x"]
    B, S, C = x.shape
    Wq_eff, Wk_eff, Wv_eff, Wo_eff = fold_weights(
        inputs["Wq"], inputs["Wk"], inputs["Wv"], inputs["Wo"],
        inputs["Aq"], inputs["Bq"], inputs["Ak"], inputs["Bk"],
        inputs["Av"], inputs["Bv"], inputs["Ao"], inputs["Bo"],
    )
    in_maps = make_core_inputs(x, Wq_eff, Wk_eff, Wv_eff, Wo_eff)
    nc = _get_program(S, C)
    res = run_bass_kernel_spmd(nc, in_maps, list(range(N_CORES)))
    parts = [res.results[c]["out_part"].astype(np.float32) for c in range(N_CORES)]
    bo = inputs["bo"]
    out = np.stack(
        [
            parts[0] + parts[1] + parts[2] + parts[3] + bo,
            parts[4] + parts[5] + parts[6] + parts[7] + bo,
        ]
    ).astype(np.float32)
    return out



# revision 2
# speedup vs baseline: 1.7675x; 1.7675x over previous
"""Trainium2 Bass kernel for LoRA self-attention — v4: fully-paired scores.

Like v3 (paired-scores lanes) plus:

* Head 4 now pairs with itself across q-stripes: q4 is bounced (via DRAM)
  to partitions 64:128 of qkT chunk 5, so lane A = (q4-lo, k4@chunk5-lo)
  and lane B = (q4-hi@chunk5-hi, k4@chunk4-hi) target disjoint PE
  row-groups.  All 20 (head, q-stripe) lanes run as 10 paired blocks.
* Lanes carry explicit (qc, qo, kc, ko) SBUF locations.
* PSUM: scores 2x[128,1024] + ctx 2x[128,512] + proj 2x[128,512] = 8 banks.
"""

import sys

if "/opt/trn_rl_repo" not in sys.path:
    sys.path.insert(0, "/opt/trn_rl_repo")

from contextlib import ExitStack

import ml_dtypes
import numpy as np

import concourse.bass as bass
import concourse.tile as tile
from concourse import bacc, mybir
from concourse.bass_utils import run_bass_kernel_spmd

BF16 = mybir.dt.bfloat16
F32 = mybir.dt.float32
NPBF16 = ml_dtypes.bfloat16

D = 64
H_LOC = 5
N_CORES = 8

Q_LOC = {0: (0, 0), 1: (0, 64), 2: (2, 0), 3: (2, 64), 4: (4, 0)}
K_LOC = {0: (1, 0), 1: (1, 64), 2: (3, 0), 3: (3, 64), 4: (5, 0)}


def build_program(S=2048, C=1280, repeat=1):
    assert S % 1024 == 0 and C % 128 == 0
    CK = C // 128
    SM = S // 128
    SK = S // 128
    NS4 = S // 512

    nc = bacc.Bacc("TRN2", target_bir_lowering=False, debug=False)

    xT_d = nc.dram_tensor("xT", [C, S], BF16, kind="ExternalInput").ap()
    wqk_d = nc.dram_tensor("wqk", [C, 640], BF16, kind="ExternalInput").ap()
    wvT_d = nc.dram_tensor("wvT", [C, H_LOC * D], BF16, kind="ExternalInput").ap()
    woT_d = nc.dram_tensor("woT", [384, C], BF16, kind="ExternalInput").ap()
    out_d = nc.dram_tensor("out_part", [S, C], F32, kind="ExternalOutput").ap()

    EXP = mybir.ActivationFunctionType.Exp
    MULT = mybir.AluOpType.mult

    with tile.TileContext(nc) as tc, ExitStack() as ctx:
        persist = ctx.enter_context(tc.tile_pool(name="persist", bufs=1))
        psp = ctx.enter_context(tc.tile_pool(name="ps", bufs=2, space="PSUM"))
        ptp = ctx.enter_context(tc.tile_pool(name="ptp", bufs=4))
        unp = ctx.enter_context(tc.tile_pool(name="unp", bufs=4))
        smallp = ctx.enter_context(tc.tile_pool(name="small", bufs=4))
        outp = ctx.enter_context(tc.tile_pool(name="osb", bufs=2))
        dramp = ctx.enter_context(tc.tile_pool(name="scratch", bufs=2, space="DRAM"))

        xT_sb = persist.tile([128, CK, S], BF16, tag="xT")
        wqk_sb = persist.tile([128, CK, 640], BF16, tag="wqk")
        wvT_sb = persist.tile([128, CK, H_LOC * D], BF16, tag="wvT")
        woT_sb = persist.tile([128, 3, C], BF16, tag="woT")
        qkT_sb = persist.tile([128, 6, S], BF16, tag="qkT")
        v_sb = persist.tile([128, SM, H_LOC, D + 1], BF16, tag="vsb")
        ctxT_sb = persist.tile([128, 3, S], BF16, tag="ctxT")

        def emit_body(rep):
            for s4 in range(NS4):
                nc.sync.dma_start(
                    xT_sb[:, :, s4 * 512 : (s4 + 1) * 512],
                    xT_d[:, s4 * 512 : (s4 + 1) * 512].rearrange(
                        "(o p) n -> p o n", p=128
                    ),
                )
            nc.gpsimd.dma_start(wqk_sb[:], wqk_d.rearrange("(o p) n -> p o n", p=128))
            nc.gpsimd.dma_start(wvT_sb[:], wvT_d.rearrange("(o p) n -> p o n", p=128))
            nc.gpsimd.dma_start(woT_sb[:], woT_d.rearrange("(o p) n -> p o n", p=128))

            nc.vector.memset(v_sb[:, :, :, D : D + 1], 1.0)
            if rep == 0:
                nc.vector.memset(ctxT_sb[64:128, 2, :], 0.0)

            # ---- PE work quanta -----------------------------------------
            def a1_quantum(f, s4):
                ps = psp.tile([128, 512], F32, tag="proj", name=f"a1ps_{f}_{s4}")
                for c in range(CK):
                    nc.tensor.matmul(
                        ps[:],
                        lhsT=wqk_sb[:, c, f * 128 : (f + 1) * 128],
                        rhs=xT_sb[:, c, s4 * 512 : (s4 + 1) * 512],
                        start=(c == 0),
                        stop=(c == CK - 1),
                    )
                nc.vector.tensor_copy(
                    out=qkT_sb[:, f, s4 * 512 : (s4 + 1) * 512], in_=ps[:]
                )
                if f == 4:
                    # realign k4 to partitions 0:64 of chunk 5 via DRAM
                    # bounce on the SP ring, which carries only small
                    # transfers (big output stores ride the gpsimd ring);
                    # also bounce q4 to partitions 64:128 of chunk 5 so h4
                    # can self-pair across q-stripes.
                    kscr = dramp.tile([64, 512], BF16, tag="kscr", name=f"kscr_{s4}")
                    nc.sync.dma_start(
                        kscr[:], qkT_sb[64:128, 4, s4 * 512 : (s4 + 1) * 512]
                    )
                    nc.sync.dma_start(
                        qkT_sb[0:64, 5, s4 * 512 : (s4 + 1) * 512], kscr[:]
                    )
                    qscr = dramp.tile([64, 512], BF16, tag="qscr", name=f"qscr_{s4}")
                    nc.sync.dma_start(
                        qscr[:], qkT_sb[0:64, 4, s4 * 512 : (s4 + 1) * 512]
                    )
                    nc.sync.dma_start(
                        qkT_sb[64:128, 5, s4 * 512 : (s4 + 1) * 512], qscr[:]
                    )

            def a2_quantum(m):
                ps = psp.tile([128, 512], F32, tag="proj", name=f"a2ps_{m}")
                for c in range(CK):
                    nc.tensor.matmul(
                        ps[:, 0 : H_LOC * D],
                        lhsT=xT_sb[:, c, m * 128 : (m + 1) * 128],
                        rhs=wvT_sb[:, c, :],
                        start=(c == 0),
                        stop=(c == CK - 1),
                    )
                nc.vector.tensor_copy(
                    out=v_sb[:, m, :, 0:D],
                    in_=ps[:, 0 : H_LOC * D].rearrange("p (h d) -> p h d", h=H_LOC),
                )

            def op_quantum(m):
                out_sb = outp.tile([128, C], F32, tag="osb", name=f"osb_{m}")
                for col0 in range(0, C, 512):
                    w = min(512, C - col0)
                    ps = psp.tile([128, 512], F32, tag="proj", name=f"ops_{m}_{col0}")
                    for j in range(3):
                        nc.tensor.matmul(
                            ps[:, 0:w],
                            lhsT=ctxT_sb[:, j, m * 128 : (m + 1) * 128],
                            rhs=woT_sb[:, j, col0 : col0 + w],
                            start=(j == 0),
                            stop=(j == 2),
                        )
                    nc.vector.tensor_copy(
                        out=out_sb[:, col0 : col0 + w], in_=ps[:, 0:w]
                    )
                # output stores ride the gpsimd ring (idle after weight loads)
                nc.gpsimd.dma_start(out_d[m * 128 : (m + 1) * 128, :], out_sb[:])

            # ---- attention block: 1-2 lanes, 16 sk slots ----------------
            def attention_block(lanes, fillers, bname):
                """lanes: list of (head, q0, qc, qo, kc, ko)."""
                W = 512 * len(lanes)
                ctxs = [
                    psp.tile(
                        [128, 512], F32, tag="ctx", name=f"ctx_{bname}_{li}"
                    )
                    for li in range(len(lanes))
                ]
                nf = len(fillers)
                fi = 0
                for sk in range(SK):
                    sc = psp.tile([128, 1024], F32, tag="sc", name=f"sc_{bname}_{sk}")
                    for li, (h, q0, qc, qo, kc, ko) in enumerate(lanes):
                        nc.tensor.matmul(
                            sc[:, li * 512 : (li + 1) * 512],
                            lhsT=qkT_sb[ko : ko + D, kc, sk * 128 : (sk + 1) * 128],
                            rhs=qkT_sb[qo : qo + D, qc, q0 : q0 + 512],
                            start=True,
                            stop=True,
                        )
                    pt = ptp.tile([128, 1024], BF16, tag="pt", name=f"pt_{bname}_{sk}")
                    nc.scalar.activation(pt[:, 0:W], sc[:, 0:W], EXP)
                    fto = (sk + 1) * nf // SK
                    while fi < fto:
                        fillers[fi]()
                        fi += 1
                    for li, (h, q0, qc, qo, kc, ko) in enumerate(lanes):
                        nc.tensor.matmul(
                            ctxs[li][0 : D + 1, :],
                            lhsT=v_sb[:, sk, h, :],
                            rhs=pt[:, li * 512 : (li + 1) * 512],
                            start=(sk == 0),
                            stop=(sk == SK - 1),
                        )
                for li, (h, q0, qc, qo, kc, ko) in enumerate(lanes):
                    un = unp.tile([128, 512], F32, tag="un", name=f"un_{bname}_{li}")
                    nc.vector.tensor_copy(
                        out=un[0 : D + 1, :], in_=ctxs[li][0 : D + 1, :]
                    )
                    rec = smallp.tile([1, 512], F32, tag="rec", name=f"rec_{bname}_{li}")
                    nc.vector.reciprocal(rec[:], un[D : D + 1, :])
                    # normalize bounce on the DVE ring: it gates ctxT (and so
                    # the output-projection fillers) and must not serialize
                    # behind output stores or input loads on other rings
                    scr = dramp.tile([1, 512], F32, tag="scr", name=f"scr_{bname}_{li}")
                    nc.sync.dma_start(scr[:], rec[:])
                    bc = smallp.tile([64, 512], F32, tag="bc", name=f"bc_{bname}_{li}")
                    nc.sync.dma_start(bc[:], scr[:].to_broadcast((64, 512)))
                    jc, po = h // 2, (h % 2) * 64
                    nc.vector.tensor_tensor(
                        out=ctxT_sb[po : po + D, jc, q0 : q0 + 512],
                        in0=un[0:D, :],
                        in1=bc[:],
                        op=MULT,
                    )

            # ---- schedule ----------------------------------------------
            a1_quantum(0, 0)
            a1_quantum(1, 0)
            a2_quantum(0)
            a2_quantum(1)
            a2_quantum(2)
            a2_quantum(3)

            A1 = lambda f, s: (lambda: a1_quantum(f, s))
            A2 = lambda m: (lambda: a2_quantum(m))
            OP = lambda m: (lambda: op_quantum(m))

            def LN(h, q0):
                qc, qo = Q_LOC[h]
                kc, ko = K_LOC[h]
                return (h, q0, qc, qo, kc, ko)

            # h4 hi-copy lane: q4 bounced to chunk5[64:], k4 original chunk4[64:]
            def LN4HI(q0):
                return (4, q0, 5, 64, 4, 64)

            blocks = [
                ([LN(0, 0), LN(1, 0)],
                 [A1(1, 1), A1(1, 2), A1(1, 3)]
                 + [A2(m) for m in range(4, 16)] + [A1(0, 1)], "p0s0"),
                ([LN(0, 512), LN(1, 512)],
                 [A1(0, 2), A1(3, 0), A1(3, 1), A1(3, 2)], "p0s1"),
                ([LN(0, 1024), LN(1, 1024)],
                 [A1(0, 3), A1(3, 3), A1(2, 0), A1(2, 1)], "p0s2"),
                ([LN(2, 0), LN(3, 0)], [A1(2, 2), A1(2, 3), A1(4, 0)], "p1s0"),
                ([LN(2, 512), LN(3, 512)], [A1(4, 1), A1(4, 2), A1(4, 3)], "p1s1"),
                ([LN(4, 0), LN4HI(512)], [], "h4a"),
                # q 0:1024 done for all heads -> op m0..7 ready
                ([LN(2, 1024), LN(3, 1024)], [OP(0), OP(1), OP(2)], "p1s2"),
                ([LN(4, 1024), LN4HI(1536)], [OP(3), OP(4), OP(5)], "h4b"),
                ([LN(0, 1536), LN(1, 1536)], [OP(6), OP(7), OP(8)], "p0s3"),
                ([LN(2, 1536), LN(3, 1536)], [OP(9), OP(10), OP(11)], "p1s3"),
            ]
            for lanes, fillers, bname in blocks:
                attention_block(lanes, fillers, bname)
            for m in range(12, 16):
                op_quantum(m)

        for rep in range(repeat):
            emit_body(rep)

    nc.compile()
    return nc


def make_core_inputs(x, Wq_eff, Wk_eff, Wv_eff, Wo_eff):
    B, S, C = x.shape
    in_maps = []
    xT16 = [np.ascontiguousarray(x[b].T).astype(NPBF16) for b in range(B)]
    for core in range(N_CORES):
        b, g = core // 4, core % 4
        r0 = g * H_LOC * D
        qf = Wq_eff[r0 : r0 + H_LOC * D]
        kf = Wk_eff[r0 : r0 + H_LOC * D]
        vf = Wv_eff[r0 : r0 + H_LOC * D]
        wqk = np.concatenate(
            [
                qf[0:128], kf[0:128],
                qf[128:256], kf[128:256],
                qf[256:320], kf[256:320],
            ],
            axis=0,
        ).T
        wvT = vf.T
        woT = np.concatenate(
            [Wo_eff[:, r0 : r0 + H_LOC * D].T, np.zeros((D, C), np.float32)], axis=0
        )
        in_maps.append(
            {
                "xT": xT16[b],
                "wqk": np.ascontiguousarray(wqk).astype(NPBF16),
                "wvT": np.ascontiguousarray(wvT).astype(NPBF16),
                "woT": np.ascontiguousarray(woT).astype(NPBF16),
            }
        )
    return in_maps


def fold_weights(Wq, Wk, Wv, Wo, Aq, Bq, Ak, Bk, Av, Bv, Ao, Bo):
    scale = 1.0 / np.sqrt(np.float32(D))
    Wq_eff = (Wq + Bq @ Aq) * scale
    Wk_eff = Wk + Bk @ Ak
    Wv_eff = Wv + Bv @ Av
    Wo_eff = Wo + Bo @ Ao
    return Wq_eff, Wk_eff, Wv_eff, Wo_eff


_NC_CACHE = {}


def _get_program(S, C):
    key = (S, C)
    if key not in _NC_CACHE:
        _NC_CACHE[key] = build_program(S, C)
    return _NC_CACHE[key]


def kernel(**inputs):
    inputs = {k: np.asarray(v, np.float32) for k, v in inputs.items()}
    x = inputs["x"]
    B, S, C = x.shape
    Wq_eff, Wk_eff, Wv_eff, Wo_eff = fold_weights(
        inputs["Wq"], inputs["Wk"], inputs["Wv"], inputs["Wo"],
        inputs["Aq"], inputs["Bq"], inputs["Ak"], inputs["Bk"],
        inputs["Av"], inputs["Bv"], inputs["Ao"], inputs["Bo"],
    )
    in_maps = make_core_inputs(x, Wq_eff, Wk_eff, Wv_eff, Wo_eff)
    nc = _get_program(S, C)
    res = run_bass_kernel_spmd(nc, in_maps, list(range(N_CORES)))
    parts = [res.results[c]["out_part"].astype(np.float32) for c in range(N_CORES)]
    bo = inputs["bo"]
    out = np.stack(
        [
            parts[0] + parts[1] + parts[2] + parts[3] + bo,
            parts[4] + parts[5] + parts[6] + parts[7] + bo,
        ]
    ).astype(np.float32)
    return out

